# revision 5
# baseline (speedup 1.0000x reference)
"""Channel-attention (CAM) Bass kernel for TRN2, SPMD over 8 NeuronCores.

Computes, for each batch b:
    A   = inputs[b].reshape(HW, C)
    G   = A.T @ A                      (Gram, [C, C])
    S   = softmax(G, axis=-1)
    out = gamma * (A @ S) + A

Sharding: data-parallel over batch. 16 batches / 8 cores = 2 batches per core.

Numerics: the epilogue is computed in residual form
    out = A @ (gamma*S - gamma*I) + (1 + gamma) * A
which is algebraically identical but applies the identity component of S to
the exact fp32 copy of A, so the fp8 matmul precision only touches the
gamma*(S - I) term.

v2 design (fp8 DoubleRow):
  - Row-pair layout: SBUF chunk [128, 2, 512] holds rows 256t + 2p + r on
    partition p, giving 4 KiB contiguous HBM descriptors per partition
    (doubles effective DMA packet size) and mapping directly onto the fp8
    DoubleRow k-subtile pair layout [part, 2, free].
  - A cast to fp8e4 (ScalarE/GpSimd alternating); Gram runs as one
    DoubleRow matmul per (chunk, m): contracts 256 rows/instr at 2x rate.
  - A^T for the attend is built with "dual transposes": a regular fp8
    DoubleRow matmul against a block-diagonal double identity constant
    transposes two adjacent 128x128 blocks per instruction.
  - Softmax: DVE row-max (negated) -> ScalarE Exp with accum_out row-sum
    -> DVE reciprocal -> scale by gamma -> S'' = (E * gamma*r) - gamma*I
    written as fp8e4 into the DoubleRow-paired S2 tiles.
  - Attend: DoubleRow matmuls, stationary = A^T pair slice, moving = S2.
  - Epilogue: out = psum + (1+gamma)*raw in one scalar_tensor_tensor per
    r-half, split across DVE (r=0) and GpSimd (r=1); DMA out per chunk.
"""

import numpy as np

import concourse.bass as bass
import concourse.mybir as mybir
import concourse.tile as tile
from concourse import bacc
from concourse.bass import ds, ts
from concourse.masks import make_identity

P = 128
N_CORES = 8
B_TOTAL = 16
B_PER_CORE = B_TOTAL // N_CORES  # 2
H = 64
W = 64
HW = H * W          # 4096
C = 512
R = 2               # rows per partition (DoubleRow pair)
NT = HW // (P * R)  # 16 chunks of 256 rows per batch
M = C // P          # 4 channel chunks
MP = M // 2         # 2 channel-chunk pairs
DEFER_T = 3         # chunks whose dual-transposes run during the softmax

F32 = mybir.dt.float32
BF16 = mybir.dt.bfloat16
FP8 = mybir.dt.float8e4
AX = mybir.AxisListType
ALU = mybir.AluOpType
ACT_FN = mybir.ActivationFunctionType
DR = mybir.MatmulPerfMode.DoubleRow


def _build_kernel(tc, a_dram, gamma_dram, o_dram):
    nc = tc.nc
    from contextlib import ExitStack

    with ExitStack() as ctx:
        const_pool = ctx.enter_context(tc.tile_pool(name="const", bufs=1))
        raw_pool = ctx.enter_context(tc.tile_pool(name="raw", bufs=20))
        a8_pool = ctx.enter_context(tc.tile_pool(name="a8", bufs=6))
        at_pool = ctx.enter_context(tc.tile_pool(name="at", bufs=24))
        e_pool = ctx.enter_context(tc.tile_pool(name="e", bufs=M))
        s_pool = ctx.enter_context(tc.tile_pool(name="s", bufs=2 * MP))
        st_pool = ctx.enter_context(tc.tile_pool(name="st", bufs=16))
        o_pool = ctx.enter_context(tc.tile_pool(name="o", bufs=4))
        pg_pool = ctx.enter_context(tc.tile_pool(name="pg", bufs=M, space="PSUM"))
        pt_pool = ctx.enter_context(tc.tile_pool(name="pt", bufs=1, space="PSUM"))
        po_pool = ctx.enter_context(tc.tile_pool(name="po", bufs=1, space="PSUM"))

        # Dual identity for 2-at-a-time block transposes: I2[p, 0, f] = [f==p],
        # I2[p, 1, f] = [f==128+p]  (fp8; 1.0 is exact).
        ident2 = const_pool.tile([P, R, 2 * P], FP8, tag="ident2")
        i2flat = ident2.rearrange("p q f -> p (q f)")
        nc.gpsimd.memset(i2flat, 0.0)
        nc.gpsimd.affine_select(
            out=i2flat, in_=i2flat, compare_op=ALU.not_equal, fill=1.0,
            base=0, pattern=[[-1, 2 * R * P]], channel_multiplier=1,
        )
        nc.gpsimd.affine_select(
            out=i2flat, in_=i2flat, compare_op=ALU.not_equal, fill=1.0,
            base=3 * P, pattern=[[-1, 2 * R * P]], channel_multiplier=1,
        )

        gamma_sb = const_pool.tile([P, 1], F32, tag="gamma")
        nc.sync.dma_start(gamma_sb, gamma_dram)
        gamma2_sb = const_pool.tile([P, 1], F32, tag="gamma2")
        nc.vector.tensor_scalar_add(gamma2_sb, gamma_sb, 1.0)
        # identrow[m]: gamma * I placed at columns [128m, 128m+128) of a
        # [128, 512] row block, fp32
        identrow = []
        for m in range(M):
            ir = const_pool.tile([P, C], F32, tag=f"identrow{m}", name="ir")
            nc.gpsimd.memset(ir, 0.0)
            make_identity(nc, ir[:, ts(m, P)], nomemset=True)
            nc.vector.tensor_scalar_mul(ir, ir, gamma_sb)
            identrow.append(ir)

        for b in range(B_PER_CORE):
            # row = 256*t + 2*p + r on partition p
            a_b = a_dram[b].rearrange("(t p r) c -> p t r c", p=P, r=R)
            o_b = o_dram[b].rearrange("(t p r) c -> p t r c", p=P, r=R)

            raw = []
            a8 = []      # [P, R, M, P] fp8 views
            at = []      # [P, MP, R, 2, P] fp8
            g_ps = [pg_pool.tile([P, C], F32, tag="pg", name="g_ps") for m in range(M)]

            def do_dual_transposes(t2):
                pt = pt_pool.tile([P, MP, R, 2 * P], F32, tag="pt", name="pt")
                for mp in range(MP):
                    for r in range(R):
                        nc.tensor.matmul(
                            pt[:, mp, r, :],
                            a8[t2][:, r, ts(mp, 2), :],
                            ident2,
                            start=True,
                            stop=True,
                            perf_mode=DR,
                        )
                att = at_pool.tile([P, MP, R, 2, P], FP8, tag="at", name="att")
                atv = att.rearrange("p mp r q n -> p (mp r q n)")
                ptv = pt.rearrange("p mp r f -> p (mp r f)")
                half = R * 2 * P
                nc.vector.tensor_copy(out=atv[:, :half], in_=ptv[:, :half])
                nc.scalar.activation(
                    atv[:, half:], ptv[:, half:], ACT_FN.Copy, bias=0.0, scale=1.0
                )
                at.append(att)

            for t2 in range(NT):
                rawt = raw_pool.tile([P, R, C], F32, tag="raw", name="rawt")
                if t2 == 0:
                    for r in range(R):
                        nc.sync.dma_start(rawt[:, r, :], a_b[:, t2, r, :])
                else:
                    nc.sync.dma_start(rawt, a_b[:, t2])
                raw.append(rawt)

                a8t = a8_pool.tile([P, R, M, P], FP8, tag="a8", name="a8t")
                cast_eng = nc.scalar if t2 % 2 == 0 else nc.gpsimd
                a8v = a8t.rearrange("p r m n -> p (r m n)")
                rawv = rawt.rearrange("p r c -> p (r c)")
                if t2 == 0:
                    nc.scalar.activation(
                        a8v[:, : M * P], rawv[:, : M * P], ACT_FN.Copy,
                        bias=0.0, scale=1.0,
                    )
                    nc.scalar.activation(
                        a8v[:, M * P:], rawv[:, M * P:], ACT_FN.Copy,
                        bias=0.0, scale=1.0,
                    )
                elif cast_eng is nc.scalar:
                    nc.scalar.activation(a8v, rawv, ACT_FN.Copy, bias=0.0, scale=1.0)
                else:
                    nc.gpsimd.tensor_copy(out=a8v, in_=rawv)
                a8.append(a8t)

                # Gram: one DoubleRow matmul per m, contracting this chunk's
                # 256 rows
                for m in range(M):
                    nc.tensor.matmul(
                        g_ps[m],
                        a8t[:, :, m, :],
                        a8t.rearrange("p r m n -> p r (m n)"),
                        start=(t2 == 0),
                        stop=(t2 == NT - 1),
                        perf_mode=DR,
                    )
                if t2 < NT - DEFER_T:
                    do_dual_transposes(t2)

            # Row softmax of G -> S'' = gamma*S - gamma*I, fp8, packed as
            # DoubleRow pairs: s2[mp][:, q, :] = S''[(2mp+q)*128 : ..., :]
            s2 = [
                s_pool.tile([P, R, C], FP8, tag="s2", name="s2") for mp in range(MP)
            ]
            for m in range(M):
                negmax = st_pool.tile([P, 1], F32, tag="stat", name="negmax")
                nc.vector.tensor_reduce(
                    negmax, g_ps[m], axis=AX.X, op=ALU.max, negate=True
                )
                e = e_pool.tile([P, C], F32, tag="e", name="e")
                dsum = st_pool.tile([P, 1], F32, tag="stat", name="dsum")
                nc.scalar.activation(
                    e, g_ps[m], ACT_FN.Exp, bias=negmax, scale=1.0, accum_out=dsum
                )
                r_ = st_pool.tile([P, 1], F32, tag="stat", name="r")
                nc.vector.reciprocal(r_, dsum)
                r2 = st_pool.tile([P, 1], F32, tag="stat", name="r2")
                nc.vector.tensor_scalar_mul(r2, r_, gamma_sb)
                nc.vector.scalar_tensor_tensor(
                    s2[m // 2][:, m % 2, :], e, r2, identrow[m],
                    op0=ALU.mult, op1=ALU.subtract,
                )

            for t2 in range(max(NT - DEFER_T, 0), NT):
                do_dual_transposes(t2)

            # Attend (A @ S'') + residual epilogue
            for t2 in range(NT):
                o_ps = po_pool.tile([P, R, C], F32, tag="po", name="o_ps")
                for r in range(R):
                    for mp in range(MP):
                        nc.tensor.matmul(
                            o_ps[:, r, :],
                            at[t2][:, mp, r, :, :],
                            s2[mp],
                            start=(mp == 0),
                            stop=(mp == MP - 1),
                            perf_mode=DR,
                        )
                o_sb = o_pool.tile([P, R, C], F32, tag="o", name="o_sb")
                nc.vector.scalar_tensor_tensor(
                    o_sb.rearrange("p r c -> p (r c)"),
                    raw[t2].rearrange("p r c -> p (r c)"),
                    gamma2_sb,
                    o_ps.rearrange("p r c -> p (r c)"),
                    op0=ALU.mult, op1=ALU.add,
                )
                nc.sync.dma_start(o_b[:, t2], o_sb)


_NC_CACHE = None


def build():
    global _NC_CACHE
    if _NC_CACHE is not None:
        return _NC_CACHE
    nc = bacc.Bacc(
        "TRN2",
        target_bir_lowering=False,
        debug=False,
        enable_asserts=False,
        num_devices=N_CORES,
    )
    a_dram = nc.dram_tensor("a", [B_PER_CORE, HW, C], F32, kind="ExternalInput").ap()
    gamma_dram = nc.dram_tensor("gamma", [P, 1], F32, kind="ExternalInput").ap()
    o_dram = nc.dram_tensor("o", [B_PER_CORE, HW, C], F32, kind="ExternalOutput").ap()
    with tile.TileContext(nc) as tc:
        _build_kernel(tc, a_dram, gamma_dram, o_dram)
    nc.compile()
    _NC_CACHE = nc
    return nc


def make_in_maps(inputs, gamma):
    x = np.ascontiguousarray(np.asarray(inputs, dtype=np.float32)).reshape(
        B_TOTAL, HW, C
    )
    gb = np.ascontiguousarray(
        np.broadcast_to(np.asarray(gamma, dtype=np.float32).reshape(1, 1), (P, 1))
    )
    return [
        {"a": x[i * B_PER_CORE : (i + 1) * B_PER_CORE], "gamma": gb}
        for i in range(N_CORES)
    ]


def run(inputs, gamma, trace=False, **kw):
    from concourse import bass_utils

    nc = build()
    in_maps = make_in_maps(inputs, gamma)
    res = bass_utils.run_bass_kernel_spmd(
        nc, in_maps, core_ids=list(range(N_CORES)), trace=trace, **kw
    )
    out = np.concatenate([r["o"] for r in res.results], axis=0)
    return out.reshape(B_TOTAL, H, W, C).astype(np.float32, copy=False), res


def kernel(inputs, gamma):
    out, _ = run(inputs, gamma, trace=False)
    return out


# revision 6
# speedup vs baseline: 1.4073x; 1.4073x over previous
"""Channel-attention (CAM) Bass kernel for TRN2, SPMD over 8 NeuronCores.

Computes, for each batch b:
    A   = inputs[b].reshape(HW, C)
    G   = A.T @ A                      (Gram, [C, C])
    S   = softmax(G, axis=-1)
    out = gamma * (A @ S) + A

Sharding: data-parallel over batch. 16 batches / 8 cores = 2 batches per core.

Numerics: the epilogue is computed in residual form
    out = A @ (gamma*S - gamma*I) + (1 + gamma) * A
which is algebraically identical but applies the identity component of S to
the exact fp32 copy of A, so the fp8 matmul precision only touches the
gamma*(S - I) term.

v3 design (fp8 DoubleRow, software-pipelined across the 2 batches):
  - Row-pair layout: SBUF chunk [128, 2, 512] holds rows 256t + 2p + r on
    partition p, giving 4 KiB contiguous HBM descriptors per partition
    (~400 GB/s observed vs ~325 GB/s with 2 KiB) and mapping directly onto
    the fp8 DoubleRow k-subtile pair layout [part, 2, free].
  - A cast to fp8e4 on ScalarE; Gram runs as one DoubleRow matmul per
    (chunk, m): contracts 256 rows/instr at 2x rate.  The Gram is split
    into two m-half passes (m=0,1 then m=2,3) so only 2 PSUM banks are
    live per pass and the softmax pipelines per-half.
  - A^T for the attend is built with "dual transposes": a regular fp8
    DoubleRow matmul against a block-diagonal double identity constant
    transposes two adjacent 128x128 blocks per instruction.
  - Softmax: DVE row-max (negated) -> ScalarE Exp with accum_out row-sum
    -> DVE reciprocal -> scale by gamma -> S'' = (E * gamma*r) - gamma*I
    written as fp8e4 into the DoubleRow-paired S2 tiles.
  - Attend: DoubleRow matmuls, stationary = A^T pair slice, moving = S2,
    one PSUM bank per (chunk, r), double-buffered.
  - Epilogue: out = psum + (1+gamma)*raw in one scalar_tensor_tensor per
    r-half on DVE; DMA out per chunk.
  - Cross-batch pipeline: batch1's load+pass1 Gram is interleaved into
    batch0's pass2 loop, and batch1's pass2 into batch0's attend loop,
    keeping the PE dense (max p-state) and the DMA engines streaming.
"""

import numpy as np

import concourse.bass as bass
import concourse.mybir as mybir
import concourse.tile as tile
from concourse import bacc
from concourse.bass import ds, ts
from concourse.masks import make_identity

P = 128
N_CORES = 8
B_TOTAL = 16
B_PER_CORE = B_TOTAL // N_CORES  # 2
H = 64
W = 64
HW = H * W          # 4096
C = 512
R = 2               # rows per partition (DoubleRow pair)
NT = HW // (P * R)  # 16 chunks of 256 rows per batch
M = C // P          # 4 channel chunks
MP = M // 2         # 2 channel-chunk pairs

F32 = mybir.dt.float32
BF16 = mybir.dt.bfloat16
FP8 = mybir.dt.float8e4
AX = mybir.AxisListType
ALU = mybir.AluOpType
ACT_FN = mybir.ActivationFunctionType
DR = mybir.MatmulPerfMode.DoubleRow


class _BatchState:
    __slots__ = ("a_b", "o_b", "raw", "a8", "at", "g_ps", "s2")

    def __init__(self, a_b, o_b):
        self.a_b = a_b
        self.o_b = o_b
        self.raw = []
        self.a8 = []
        self.at = []
        self.g_ps = {}
        self.s2 = None


def _build_kernel(tc, a_dram, gamma_dram, o_dram):
    nc = tc.nc
    from contextlib import ExitStack

    with ExitStack() as ctx:
        const_pool = ctx.enter_context(tc.tile_pool(name="const", bufs=1))
        raw_pool = ctx.enter_context(tc.tile_pool(name="raw", bufs=24))
        a8_pool = ctx.enter_context(tc.tile_pool(name="a8", bufs=18))
        at_pool = ctx.enter_context(tc.tile_pool(name="at", bufs=18))
        e_pool = ctx.enter_context(tc.tile_pool(name="e", bufs=M))
        s_pool = ctx.enter_context(tc.tile_pool(name="s", bufs=2 * MP))
        st_pool = ctx.enter_context(tc.tile_pool(name="st", bufs=16))
        o_pool = ctx.enter_context(tc.tile_pool(name="o", bufs=4))
        pg_pool = ctx.enter_context(tc.tile_pool(name="pg", bufs=4, space="PSUM"))
        pt_pool = ctx.enter_context(tc.tile_pool(name="pt", bufs=2, space="PSUM"))
        po_pool = ctx.enter_context(tc.tile_pool(name="po", bufs=2, space="PSUM"))

        # Dual identity for 2-at-a-time block transposes: I2[p, 0, f] = [f==p],
        # I2[p, 1, f] = [f==128+p]  (fp8; 1.0 is exact).
        ident2 = const_pool.tile([P, R, 2 * P], FP8, tag="ident2")
        i2flat = ident2.rearrange("p q f -> p (q f)")
        nc.gpsimd.memset(i2flat, 0.0)
        nc.gpsimd.affine_select(
            out=i2flat, in_=i2flat, compare_op=ALU.not_equal, fill=1.0,
            base=0, pattern=[[-1, 2 * R * P]], channel_multiplier=1,
        )
        nc.gpsimd.affine_select(
            out=i2flat, in_=i2flat, compare_op=ALU.not_equal, fill=1.0,
            base=3 * P, pattern=[[-1, 2 * R * P]], channel_multiplier=1,
        )

        gamma_sb = const_pool.tile([P, 1], F32, tag="gamma")
        nc.sync.dma_start(gamma_sb, gamma_dram)
        gamma2_sb = const_pool.tile([P, 1], F32, tag="gamma2")
        nc.vector.tensor_scalar_add(gamma2_sb, gamma_sb, 1.0)
        # identrow[m]: gamma * I placed at columns [128m, 128m+128) of a
        # [128, 512] row block, fp32
        identrow = []
        for m in range(M):
            ir = const_pool.tile([P, C], F32, tag=f"identrow{m}", name="ir")
            nc.gpsimd.memset(ir, 0.0)
            make_identity(nc, ir[:, ts(m, P)], nomemset=True)
            nc.vector.tensor_scalar_mul(ir, ir, gamma_sb)
            identrow.append(ir)

        bs = []
        for b in range(B_PER_CORE):
            # row = 256*t + 2*p + r on partition p
            bs.append(
                _BatchState(
                    a_dram[b].rearrange("(t p r) c -> p t r c", p=P, r=R),
                    o_dram[b].rearrange("(t p r) c -> p t r c", p=P, r=R),
                )
            )

        # ---- emission helpers -------------------------------------------

        def load_chunk(s, t2):
            rawt = raw_pool.tile([P, R, C], F32, tag="raw", name="rawt")
            if t2 == 0:
                for r in range(R):
                    nc.sync.dma_start(rawt[:, r, :], s.a_b[:, t2, r, :])
            else:
                nc.sync.dma_start(rawt, s.a_b[:, t2])
            s.raw.append(rawt)

            a8t = a8_pool.tile([P, R, M, P], FP8, tag="a8", name="a8t")
            a8v = a8t.rearrange("p r m n -> p (r m n)")
            rawv = rawt.rearrange("p r c -> p (r c)")
            if t2 == 0:
                half = M * P
                nc.scalar.activation(
                    a8v[:, :half], rawv[:, :half], ACT_FN.Copy, bias=0.0, scale=1.0
                )
                nc.scalar.activation(
                    a8v[:, half:], rawv[:, half:], ACT_FN.Copy, bias=0.0, scale=1.0
                )
            else:
                nc.scalar.activation(a8v, rawv, ACT_FN.Copy, bias=0.0, scale=1.0)
            s.a8.append(a8t)

        def gram_chunk(s, t2, ms):
            for m in ms:
                nc.tensor.matmul(
                    s.g_ps[m],
                    s.a8[t2][:, :, m, :],
                    s.a8[t2].rearrange("p r m n -> p r (m n)"),
                    start=(t2 == 0),
                    stop=(t2 == NT - 1),
                    perf_mode=DR,
                )

        def alloc_gram(s, ms):
            for m in ms:
                s.g_ps[m] = pg_pool.tile([P, C], F32, tag="pg", name="g_ps")

        def dual_transposes(s, t2, copy_engines):
            att = at_pool.tile([P, MP, R, 2, P], FP8, tag="at", name="att")
            for mp in range(MP):
                pt = pt_pool.tile([P, R, 2 * P], F32, tag="pt", name="pt")
                for r in range(R):
                    nc.tensor.matmul(
                        pt[:, r, :],
                        s.a8[t2][:, r, ts(mp, 2), :],
                        ident2,
                        start=True,
                        stop=True,
                        perf_mode=DR,
                    )
                dst = att[:, mp].rearrange("p r q n -> p (r q n)")
                src = pt.rearrange("p r f -> p (r f)")
                eng = copy_engines[mp]
                if eng == "v":
                    nc.vector.tensor_copy(out=dst, in_=src)
                else:
                    nc.scalar.activation(dst, src, ACT_FN.Copy, bias=0.0, scale=1.0)
            s.at.append(att)

        def softmax_half(s, half):
            if s.s2 is None:
                s.s2 = [
                    s_pool.tile([P, R, C], FP8, tag="s2", name="s2")
                    for _ in range(MP)
                ]
            for m in (2 * half, 2 * half + 1):
                negmax = st_pool.tile([P, 1], F32, tag="stat", name="negmax")
                nc.vector.tensor_reduce(
                    negmax, s.g_ps[m], axis=AX.X, op=ALU.max, negate=True
                )
                e = e_pool.tile([P, C], F32, tag="e", name="e")
                dsum = st_pool.tile([P, 1], F32, tag="stat", name="dsum")
                nc.scalar.activation(
                    e, s.g_ps[m], ACT_FN.Exp, bias=negmax, scale=1.0, accum_out=dsum
                )
                r_ = st_pool.tile([P, 1], F32, tag="stat", name="r")
                nc.vector.reciprocal(r_, dsum)
                r2 = st_pool.tile([P, 1], F32, tag="stat", name="r2")
                nc.vector.tensor_scalar_mul(r2, r_, gamma_sb)
                nc.vector.scalar_tensor_tensor(
                    s.s2[m // 2][:, m % 2, :], e, r2, identrow[m],
                    op0=ALU.mult, op1=ALU.subtract,
                )

        def attend_chunk(s, t2):
            o_sb = o_pool.tile([P, R, C], F32, tag="o", name="o_sb")
            for r in range(R):
                o_ps = po_pool.tile([P, C], F32, tag="po", name="o_ps")
                for mp in range(MP):
                    nc.tensor.matmul(
                        o_ps,
                        s.at[t2][:, mp, r, :, :],
                        s.s2[mp],
                        start=(mp == 0),
                        stop=(mp == MP - 1),
                        perf_mode=DR,
                    )
                nc.vector.scalar_tensor_tensor(
                    o_sb[:, r, :], s.raw[t2][:, r, :], gamma2_sb, o_ps,
                    op0=ALU.mult, op1=ALU.add,
                )
            nc.sync.dma_start(s.o_b[:, t2], o_sb)

        # ---- schedule ----------------------------------------------------

        b0, b1 = bs

        # Phase 1: load batch0, Gram m=0,1
        alloc_gram(b0, (0, 1))
        for t2 in range(NT):
            load_chunk(b0, t2)
            gram_chunk(b0, t2, (0, 1))
        softmax_half(b0, 0)

        # Phase 2: batch0 pass2 (dual transposes + Gram m=2,3), interleaved
        # with batch1's load + Gram m=0,1
        alloc_gram(b0, (2, 3))
        alloc_gram(b1, (0, 1))
        for t2 in range(NT):
            dual_transposes(b0, t2, ("v", "s"))
            gram_chunk(b0, t2, (2, 3))
            load_chunk(b1, t2)
            gram_chunk(b1, t2, (0, 1))
        softmax_half(b0, 1)
        softmax_half(b1, 0)

        # Phase 3: batch0 attend, interleaved with batch1 pass2
        alloc_gram(b1, (2, 3))
        for t2 in range(NT):
            attend_chunk(b0, t2)
            dual_transposes(b1, t2, ("s", "s"))
            gram_chunk(b1, t2, (2, 3))
        softmax_half(b1, 1)

        # Phase 4: batch1 attend
        for t2 in range(NT):
            attend_chunk(b1, t2)


_NC_CACHE = None


def build():
    global _NC_CACHE
    if _NC_CACHE is not None:
        return _NC_CACHE
    nc = bacc.Bacc(
        "TRN2",
        target_bir_lowering=False,
        debug=False,
        enable_asserts=False,
        num_devices=N_CORES,
    )
    a_dram = nc.dram_tensor("a", [B_PER_CORE, HW, C], F32, kind="ExternalInput").ap()
    gamma_dram = nc.dram_tensor("gamma", [P, 1], F32, kind="ExternalInput").ap()
    o_dram = nc.dram_tensor("o", [B_PER_CORE, HW, C], F32, kind="ExternalOutput").ap()
    with tile.TileContext(nc) as tc:
        _build_kernel(tc, a_dram, gamma_dram, o_dram)
    nc.compile()
    _NC_CACHE = nc
    return nc


def make_in_maps(inputs, gamma):
    x = np.ascontiguousarray(np.asarray(inputs, dtype=np.float32)).reshape(
        B_TOTAL, HW, C
    )
    gb = np.ascontiguousarray(
        np.broadcast_to(np.asarray(gamma, dtype=np.float32).reshape(1, 1), (P, 1))
    )
    return [
        {"a": x[i * B_PER_CORE : (i + 1) * B_PER_CORE], "gamma": gb}
        for i in range(N_CORES)
    ]


def run(inputs, gamma, trace=False, **kw):
    from concourse import bass_utils

    nc = build()
    in_maps = make_in_maps(inputs, gamma)
    res = bass_utils.run_bass_kernel_spmd(
        nc, in_maps, core_ids=list(range(N_CORES)), trace=trace, **kw
    )
    out = np.concatenate([r["o"] for r in res.results], axis=0)
    return out.reshape(B_TOTAL, H, W, C).astype(np.float32, copy=False), res


def kernel(inputs, gamma):
    out, _ = run(inputs, gamma, trace=False)
    return out


# revision 8
# speedup vs baseline: 1.4431x; 1.0255x over previous
"""Channel-attention (CAM) Bass kernel for TRN2, SPMD over 8 NeuronCores.

Computes, for each batch b:
    A   = inputs[b].reshape(HW, C)
    G   = A.T @ A                      (Gram, [C, C])
    S   = softmax(G, axis=-1)
    out = gamma * (A @ S) + A

Sharding: data-parallel over batch. 16 batches / 8 cores = 2 batches per core.

Numerics: the epilogue is computed in residual form
    out = A @ (gamma*S - gamma*I) + (1 + gamma) * A
which is algebraically identical but applies the identity component of S to
the exact fp32 copy of A, so the fp8 matmul precision only touches the
gamma*(S - I) term.

v4 design (fp8 DoubleRow, staggered 2-batch pipeline):
  - Row-pair layout: SBUF chunk [128, 2, 512] holds rows 256t + 2p + r on
    partition p; chunks are loaded/stored two at a time (2 MiB DMAs with
    8 KiB contiguous descriptors per partition).
  - A cast to fp8e4 on ScalarE; Gram runs as one DoubleRow matmul per
    (chunk, m) contracting 256 rows at 2x rate, split into two m-half
    passes so only 2 PSUM banks are live per pass.
  - A^T for the attend is built with "dual transposes": a regular fp8
    DoubleRow matmul against a block-diagonal double identity constant
    transposes two adjacent 128x128 blocks per instruction.
  - Softmax: DVE row-max (negated) -> ScalarE Exp with accum_out row-sum
    -> DVE reciprocal -> scale by gamma -> S'' = (E * gamma*r) - gamma*I
    written as fp8e4 into the DoubleRow-paired S2 tiles.
  - Attend: DoubleRow matmuls, one PSUM bank per (chunk, r), double
    buffered; epilogue out = psum + (1+gamma)*raw alternates between a
    direct DVE scalar_tensor_tensor and a ScalarE PSUM-drain + GpSimd
    scalar_tensor_tensor (GpSimd cannot read PSUM), spreading the fp32
    epilogue traffic over three engines.
  - Stagger: batch0's pass1+transposes overlap its own load; batch1's
    load streams during batch0's pass2/attend; batch1's pass1+transposes
    are interleaved into batch0's attend loop so the PE stays dense.
"""

import numpy as np

import concourse.bass as bass
import concourse.mybir as mybir
import concourse.tile as tile
from concourse import bacc
from concourse.bass import ds, ts
from concourse.masks import make_identity

P = 128
N_CORES = 8
B_TOTAL = 16
B_PER_CORE = B_TOTAL // N_CORES  # 2
H = 64
W = 64
HW = H * W          # 4096
C = 512
R = 2               # rows per partition (DoubleRow pair)
NT = HW // (P * R)  # 16 chunks of 256 rows per batch
NT2 = NT // 2       # 8 chunk pairs
M = C // P          # 4 channel chunks
MP = M // 2         # 2 channel-chunk pairs

F32 = mybir.dt.float32
FP8 = mybir.dt.float8e4
AX = mybir.AxisListType
ALU = mybir.AluOpType
ACT_FN = mybir.ActivationFunctionType
DR = mybir.MatmulPerfMode.DoubleRow


class _BatchState:
    __slots__ = ("a_b", "o_b", "raw2", "a8", "at", "g_ps", "s2", "o2")

    def __init__(self, a_b, o_b):
        self.a_b = a_b
        self.o_b = o_b
        self.raw2 = []   # pair tiles [P, 2, R, C]
        self.a8 = []
        self.at = []
        self.g_ps = {}
        self.s2 = None
        self.o2 = None

    def raw(self, t2):
        return self.raw2[t2 // 2][:, t2 % 2]


def _build_kernel(tc, a_dram, gamma_dram, o_dram):
    nc = tc.nc
    from contextlib import ExitStack

    with ExitStack() as ctx:
        const_pool = ctx.enter_context(tc.tile_pool(name="const", bufs=1))
        raw_pool = ctx.enter_context(tc.tile_pool(name="raw", bufs=12))
        a8_pool = ctx.enter_context(tc.tile_pool(name="a8", bufs=20))
        at_pool = ctx.enter_context(tc.tile_pool(name="at", bufs=18))
        e_pool = ctx.enter_context(tc.tile_pool(name="e", bufs=M))
        s_pool = ctx.enter_context(tc.tile_pool(name="s", bufs=2 * MP))
        st_pool = ctx.enter_context(tc.tile_pool(name="st", bufs=16))
        o_pool = ctx.enter_context(tc.tile_pool(name="o", bufs=3))
        otmp_pool = ctx.enter_context(tc.tile_pool(name="otmp", bufs=2))
        pg_pool = ctx.enter_context(tc.tile_pool(name="pg", bufs=4, space="PSUM"))
        pt_pool = ctx.enter_context(tc.tile_pool(name="pt", bufs=2, space="PSUM"))
        po_pool = ctx.enter_context(tc.tile_pool(name="po", bufs=2, space="PSUM"))

        # Dual identity for 2-at-a-time block transposes: I2[p, 0, f] = [f==p],
        # I2[p, 1, f] = [f==128+p]  (fp8; 1.0 is exact).
        ident2 = const_pool.tile([P, R, 2 * P], FP8, tag="ident2")
        i2flat = ident2.rearrange("p q f -> p (q f)")
        nc.gpsimd.memset(i2flat, 0.0)
        nc.gpsimd.affine_select(
            out=i2flat, in_=i2flat, compare_op=ALU.not_equal, fill=1.0,
            base=0, pattern=[[-1, 2 * R * P]], channel_multiplier=1,
        )
        nc.gpsimd.affine_select(
            out=i2flat, in_=i2flat, compare_op=ALU.not_equal, fill=1.0,
            base=3 * P, pattern=[[-1, 2 * R * P]], channel_multiplier=1,
        )

        gamma_sb = const_pool.tile([P, 1], F32, tag="gamma")
        nc.sync.dma_start(gamma_sb, gamma_dram)
        gamma2_sb = const_pool.tile([P, 1], F32, tag="gamma2")
        nc.vector.tensor_scalar_add(gamma2_sb, gamma_sb, 1.0)
        # identrow[m]: gamma * I placed at columns [128m, 128m+128) of a
        # [128, 512] row block, fp32
        identrow = []
        for m in range(M):
            ir = const_pool.tile([P, C], F32, tag=f"identrow{m}", name="ir")
            nc.gpsimd.memset(ir, 0.0)
            make_identity(nc, ir[:, ts(m, P)], nomemset=True)
            nc.vector.tensor_scalar_mul(ir, ir, gamma_sb)
            identrow.append(ir)

        bs = []
        for b in range(B_PER_CORE):
            # row = 256*t + 2*p + r on partition p; pairs of chunks share a DMA
            bs.append(
                _BatchState(
                    a_dram[b].rearrange("(j u p r) c -> p j u r c", p=P, r=R, u=2),
                    o_dram[b].rearrange("(j u p r) c -> p j u r c", p=P, r=R, u=2),
                )
            )

        # ---- emission helpers -------------------------------------------

        def load_pair(s, j):
            r2 = raw_pool.tile([P, 2, R, C], F32, tag="raw", name="r2")
            if j == 0:
                # finer split so the first Gram chunk starts ASAP
                for r in range(R):
                    nc.sync.dma_start(r2[:, 0, r, :], s.a_b[:, 0, 0, r, :])
                nc.sync.dma_start(r2[:, 1], s.a_b[:, 0, 1])
            else:
                nc.sync.dma_start(r2, s.a_b[:, j])
            s.raw2.append(r2)

        def cast_chunk(s, t2):
            a8t = a8_pool.tile([P, R, M, P], FP8, tag="a8", name="a8t")
            a8v = a8t.rearrange("p r m n -> p (r m n)")
            rawv = s.raw(t2).rearrange("p r c -> p (r c)")
            if t2 == 0:
                half = M * P
                nc.scalar.activation(
                    a8v[:, :half], rawv[:, :half], ACT_FN.Copy, bias=0.0, scale=1.0
                )
                nc.scalar.activation(
                    a8v[:, half:], rawv[:, half:], ACT_FN.Copy, bias=0.0, scale=1.0
                )
            else:
                nc.scalar.activation(a8v, rawv, ACT_FN.Copy, bias=0.0, scale=1.0)
            s.a8.append(a8t)

        def gram_chunk(s, t2, ms):
            for m in ms:
                nc.tensor.matmul(
                    s.g_ps[m],
                    s.a8[t2][:, :, m, :],
                    s.a8[t2].rearrange("p r m n -> p r (m n)"),
                    start=(t2 == 0),
                    stop=(t2 == NT - 1),
                    perf_mode=DR,
                )

        def alloc_gram(s, ms):
            for m in ms:
                s.g_ps[m] = pg_pool.tile([P, C], F32, tag="pg", name="g_ps")

        def dual_transposes(s, t2, copy_eng):
            att = at_pool.tile([P, MP, R, 2, P], FP8, tag="at", name="att")
            for mp in range(MP):
                pt = pt_pool.tile([P, R, 2 * P], F32, tag="pt", name="pt")
                for r in range(R):
                    nc.tensor.matmul(
                        pt[:, r, :],
                        s.a8[t2][:, r, ts(mp, 2), :],
                        ident2,
                        start=True,
                        stop=True,
                        perf_mode=DR,
                    )
                dst = att[:, mp].rearrange("p r q n -> p (r q n)")
                src = pt.rearrange("p r f -> p (r f)")
                if copy_eng == "v":
                    nc.vector.tensor_copy(out=dst, in_=src)
                else:
                    nc.scalar.activation(dst, src, ACT_FN.Copy, bias=0.0, scale=1.0)
            s.at.append(att)

        def softmax_half(s, half):
            if s.s2 is None:
                s.s2 = [
                    s_pool.tile([P, R, C], FP8, tag="s2", name="s2")
                    for _ in range(MP)
                ]
            for m in (2 * half, 2 * half + 1):
                negmax = st_pool.tile([P, 1], F32, tag="stat", name="negmax")
                nc.vector.tensor_reduce(
                    negmax, s.g_ps[m], axis=AX.X, op=ALU.max, negate=True
                )
                e = e_pool.tile([P, C], F32, tag="e", name="e")
                dsum = st_pool.tile([P, 1], F32, tag="stat", name="dsum")
                nc.scalar.activation(
                    e, s.g_ps[m], ACT_FN.Exp, bias=negmax, scale=1.0, accum_out=dsum
                )
                r_ = st_pool.tile([P, 1], F32, tag="stat", name="r")
                nc.vector.reciprocal(r_, dsum)
                r2 = st_pool.tile([P, 1], F32, tag="stat", name="r2")
                nc.vector.tensor_scalar_mul(r2, r_, gamma_sb)
                nc.vector.scalar_tensor_tensor(
                    s.s2[m // 2][:, m % 2, :], e, r2, identrow[m],
                    op0=ALU.mult, op1=ALU.subtract,
                )

        def attend_chunk(s, t2):
            if t2 % 2 == 0:
                s.o2 = o_pool.tile([P, 2, R, C], F32, tag="o", name="o2")
            o_sb = s.o2[:, t2 % 2]
            po = []
            for r in range(R):
                o_ps = po_pool.tile([P, C], F32, tag="po", name="o_ps")
                for mp in range(MP):
                    nc.tensor.matmul(
                        o_ps,
                        s.at[t2][:, mp, r, :, :],
                        s.s2[mp],
                        start=(mp == 0),
                        stop=(mp == MP - 1),
                        perf_mode=DR,
                    )
                po.append(o_ps)
            for r in range(R):
                nc.vector.scalar_tensor_tensor(
                    o_sb[:, r, :], s.raw(t2)[:, r, :], gamma2_sb, po[r],
                    op0=ALU.mult, op1=ALU.add,
                )
            if t2 % 2 == 1:
                nc.sync.dma_start(s.o_b[:, t2 // 2], s.o2)

        # ---- schedule ----------------------------------------------------

        b0, b1 = bs

        # Phase A: load batch0; Gram m=0,1 + dual transposes inline
        alloc_gram(b0, (0, 1))
        for t2 in range(NT):
            if t2 % 2 == 0:
                load_pair(b0, t2 // 2)
            cast_chunk(b0, t2)
            gram_chunk(b0, t2, (0, 1))
            dual_transposes(b0, t2, "v" if t2 % 2 == 0 else "s")
        softmax_half(b0, 0)

        # Phase B: batch1 load streams behind; batch0 Gram m=2,3 burst
        for j in range(NT2):
            load_pair(b1, j)
        alloc_gram(b0, (2, 3))
        for t2 in range(NT):
            gram_chunk(b0, t2, (2, 3))
        softmax_half(b0, 1)

        # Phase C: batch0 attend, interleaved with batch1 cast + pass1 Gram
        # + dual transposes (batch1 data arrives during this window)
        alloc_gram(b1, (0, 1))
        for t2 in range(NT):
            cast_chunk(b1, t2)
            attend_chunk(b0, t2)
            dual_transposes(b1, t2, "v" if t2 % 2 == 0 else "s")
            gram_chunk(b1, t2, (0, 1))
        softmax_half(b1, 0)

        # Phase D: batch1 Gram m=2,3 burst + softmax + attend
        alloc_gram(b1, (2, 3))
        for t2 in range(NT):
            gram_chunk(b1, t2, (2, 3))
        softmax_half(b1, 1)
        for t2 in range(NT):
            attend_chunk(b1, t2)


_NC_CACHE = None


def build():
    global _NC_CACHE
    if _NC_CACHE is not None:
        return _NC_CACHE
    nc = bacc.Bacc(
        "TRN2",
        target_bir_lowering=False,
        debug=False,
        enable_asserts=False,
        num_devices=N_CORES,
    )
    a_dram = nc.dram_tensor("a", [B_PER_CORE, HW, C], F32, kind="ExternalInput").ap()
    gamma_dram = nc.dram_tensor("gamma", [P, 1], F32, kind="ExternalInput").ap()
    o_dram = nc.dram_tensor("o", [B_PER_CORE, HW, C], F32, kind="ExternalOutput").ap()
    with tile.TileContext(nc) as tc:
        _build_kernel(tc, a_dram, gamma_dram, o_dram)
    nc.compile()
    _NC_CACHE = nc
    return nc


def make_in_maps(inputs, gamma):
    x = np.ascontiguousarray(np.asarray(inputs, dtype=np.float32)).reshape(
        B_TOTAL, HW, C
    )
    gb = np.ascontiguousarray(
        np.broadcast_to(np.asarray(gamma, dtype=np.float32).reshape(1, 1), (P, 1))
    )
    return [
        {"a": x[i * B_PER_CORE : (i + 1) * B_PER_CORE], "gamma": gb}
        for i in range(N_CORES)
    ]


def run(inputs, gamma, trace=False, **kw):
    from concourse import bass_utils

    nc = build()
    in_maps = make_in_maps(inputs, gamma)
    res = bass_utils.run_bass_kernel_spmd(
        nc, in_maps, core_ids=list(range(N_CORES)), trace=trace, **kw
    )
    out = np.concatenate([r["o"] for r in res.results], axis=0)
    return out.reshape(B_TOTAL, H, W, C).astype(np.float32, copy=False), res


def kernel(inputs, gamma):
    out, _ = run(inputs, gamma, trace=False)
    return out
